# revision 5
# baseline (speedup 1.0000x reference)
"""Single-head encoder attention block on 8 Trainium2 NeuronCores.

Math (per batch element b):
    q = x @ wq.T ; k = x @ wk.T ; v = x @ wv.T
    scores = (q @ k.T) / sqrt(1024) ; attn = softmax(scores, -1)
    out = (attn @ v) @ wo.T

Sharding: data-parallel over batch — batch 8 maps 1:1 onto the 8 cores;
weights replicated. No collectives.

Per-core algorithm (all storage fp32; matmuls run as MM_DT):
  The q/k projections are folded into one matrix M = wq.T @ wk so that no
  transpose of wq/wk is ever needed:
      scores[i,j] = x_i @ M @ x_j.T
  Phase A (LIFO pool scoping to fit SBUF):
    xT  (PE-transpose of x, resident)          [1024d, 2048s]
    { M = wq.T @ wk  (natural-layout matmuls)  [1024d1, 1024d2]
      F = (M x.T)/32 spilled to DRAM           [1024d2, 2048i]  "q-like" } scope
    { v = x @ wv.T   (resident)                [2048j, 1024vc]
      woT (PE-transpose, resident)             [1024vc, 1024do] } scope
  Phase B (per i-superblock of SB=256 query columns):
    scoresT[j,i] = sum_d2 xT[d2,j] * F[d2,i]   (xT stationary)
    expT = exp(scoresT)      (no max subtraction needed: |scores|/32 < ~3)
    rowsum over j via ones-matmul; reciprocal; transpose to per-partition
    ctxT[vc,i] = sum_j v[j,vc] expT[j,i]
    out[i,do]  = (sum_vc ctxT[vc,i] woT[vc,do]) * recip[i]
"""

import os
import sys

for _p in ("/opt/trn_rl_repo", "/root/.axon_site/_ro/trn_rl_repo"):
    if os.path.isdir(_p) and _p not in sys.path:
        sys.path.insert(0, _p)

import numpy as np
from contextlib import ExitStack

import concourse.bacc as bacc
import concourse.tile as tile
from concourse import mybir, masks
from concourse.bass_utils import run_bass_kernel_spmd

P = 128
S = 2048          # sequence length (per core)
D = 1024          # model dim = dk = dv
NS = S // P       # 16 seq tiles
ND = D // P       # 8 dim tiles
SB = 256          # i-superblock width (query columns per block)
NSB = S // SB     # 8 superblocks
SCALE = 1.0 / 32.0  # 1/sqrt(1024)
N_CORES = 8

DT = mybir.dt.float32
MM_DT = mybir.dt.float32r if os.environ.get("ENC_MM_DT", "f32r") == "f32r" else mybir.dt.float32
F32 = mybir.dt.float32
EXP = mybir.ActivationFunctionType.Exp
COPY = mybir.ActivationFunctionType.Copy


def _build():
    nc = bacc.Bacc("TRN2", target_bir_lowering=False, debug=False, num_devices=N_CORES)

    x_in = nc.dram_tensor("x", [S, D], DT, kind="ExternalInput").ap()
    wq_in = nc.dram_tensor("wq", [D, D], DT, kind="ExternalInput").ap()
    wk_in = nc.dram_tensor("wk", [D, D], DT, kind="ExternalInput").ap()
    wv_in = nc.dram_tensor("wv", [D, D], DT, kind="ExternalInput").ap()
    wo_in = nc.dram_tensor("wo", [D, D], DT, kind="ExternalInput").ap()
    out_d = nc.dram_tensor("out", [S, D], DT, kind="ExternalOutput").ap()
    f_dram = nc.dram_tensor("f_scratch", [D, S], MM_DT).ap()

    def mm(o, l, r, **kw):
        nc.tensor.matmul(o, l, r, **kw)

    with tile.TileContext(nc) as tc, ExitStack() as top:
        cst = top.enter_context(tc.tile_pool(name="cst", bufs=1))
        ident = cst.tile([P, P], DT)
        masks.make_identity(nc, ident[:])
        ones_f32 = cst.tile([P, 1], DT)
        nc.gpsimd.memset(ones_f32[:], 1.0)
        ones = cst.tile([P, 1], MM_DT)
        nc.vector.tensor_copy(ones[:], ones_f32[:])

        res1 = top.enter_context(tc.tile_pool(name="res1", bufs=1))
        xt = res1.tile([P, ND * S], MM_DT)    # xT: tile d -> [:, d*S:(d+1)*S] = [d-part, s]

        # ---------------- Phase A12: x transpose, M, F ----------------
        with ExitStack() as pa:
            tpps = pa.enter_context(tc.tile_pool(name="tpps", bufs=2, space="PSUM"))
            mmps = pa.enter_context(tc.tile_pool(name="mmps", bufs=6, space="PSUM"))
            ldp = pa.enter_context(tc.tile_pool(name="ldp", bufs=3))
            evp = pa.enter_context(tc.tile_pool(name="evp", bufs=4))
            wqk = pa.enter_context(tc.tile_pool(name="wqk", bufs=1))
            mwork = pa.enter_context(tc.tile_pool(name="mwork", bufs=1))

            # A0: load x row-tiles, PE-transpose into xT
            for s in range(NS):
                xs = ldp.tile([P, D], DT, tag="ld")
                nc.sync.dma_start(out=xs[:], in_=x_in[s * P:(s + 1) * P, :])
                for d in range(ND):
                    tp = tpps.tile([P, P], DT, tag="tp")
                    nc.tensor.transpose(tp[:], xs[:, d * P:(d + 1) * P], ident[:])
                    nc.vector.tensor_copy(xt[:, d * S + s * P: d * S + (s + 1) * P], tp[:])

            # A1: M = wq.T @ wk from natural layouts
            wqn = wqk.tile([P, ND * D], MM_DT)   # wq c-tile ct -> [:, ct*D:(ct+1)*D]
            wkn = wqk.tile([P, ND * D], MM_DT)
            mres = mwork.tile([P, ND * D], MM_DT)  # M d1-tile -> [:, d1*D:(d1+1)*D] = [d1-part, d2]
            for t in range(ND):
                nc.sync.dma_start(out=wqn[:, t * D:(t + 1) * D], in_=wq_in[t * P:(t + 1) * P, :].bitcast(MM_DT))
                nc.sync.dma_start(out=wkn[:, t * D:(t + 1) * D], in_=wk_in[t * P:(t + 1) * P, :].bitcast(MM_DT))
            for d1 in range(ND):
                for ch in range(2):
                    ps = mmps.tile([P, 512], F32, tag="mm")
                    for ct in range(ND):
                        mm(ps[:],
                           wqn[:, ct * D + d1 * P: ct * D + (d1 + 1) * P],
                           wkn[:, ct * D + ch * 512: ct * D + (ch + 1) * 512],
                           start=(ct == 0), stop=(ct == ND - 1))
                    nc.scalar.copy(mres[:, d1 * D + ch * 512: d1 * D + (ch + 1) * 512], ps[:])

            # A2: F[d2,i] = sum_d1 M[d1,d2] xT[d1,i], scaled by 1/32, spilled
            for d2 in range(ND):
                pss = [mmps.tile([P, 512], F32, name=f"fps{ic}", tag="mm") for ic in range(4)]
                for d1 in range(ND):
                    for ic in range(4):
                        mm(pss[ic][:],
                           mres[:, d1 * D + d2 * P: d1 * D + (d2 + 1) * P],
                           xt[:, d1 * S + ic * 512: d1 * S + (ic + 1) * 512],
                           start=(d1 == 0), stop=(d1 == ND - 1))
                for ic in range(4):
                    ev = evp.tile([P, 512], MM_DT, tag="ev")
                    nc.scalar.mul(ev[:], pss[ic][:], SCALE)
                    nc.sync.dma_start(out=f_dram[d2 * P:(d2 + 1) * P, ic * 512:(ic + 1) * 512], in_=ev[:])

        # residents for the attention phase (allocated after A12 freed)
        res2 = top.enter_context(tc.tile_pool(name="res2", bufs=1))
        vres = res2.tile([P, NS * D], MM_DT)  # v: tile j -> [:, j*D:(j+1)*D] = [j-part, vc]
        wot = res2.tile([P, ND * D], MM_DT)   # woT: tile vc -> [:, vc*D:(vc+1)*D] = [vc-part, do]

        # ---------------- Phase A34: v and woT ----------------
        with ExitStack() as pw:
            tpps2 = pw.enter_context(tc.tile_pool(name="tpps2", bufs=2, space="PSUM"))
            mmps2 = pw.enter_context(tc.tile_pool(name="mmps2", bufs=4, space="PSUM"))
            ldp2 = pw.enter_context(tc.tile_pool(name="ldp2", bufs=2))
            wvp = pw.enter_context(tc.tile_pool(name="wvp", bufs=1))

            wvt = wvp.tile([P, ND * D], MM_DT)  # wvT d-tile -> [:, d*D:(d+1)*D] = [d-part, vc]
            for vc in range(ND):
                wn = ldp2.tile([P, D], DT, tag="ld")
                nc.sync.dma_start(out=wn[:], in_=wv_in[vc * P:(vc + 1) * P, :])
                for d in range(ND):
                    tp = tpps2.tile([P, P], DT, tag="tp")
                    nc.tensor.transpose(tp[:], wn[:, d * P:(d + 1) * P], ident[:])
                    nc.vector.tensor_copy(wvt[:, d * D + vc * P: d * D + (vc + 1) * P], tp[:])
            for j in range(NS):
                ps2 = [mmps2.tile([P, 512], F32, name=f"vps{ch}", tag="mm") for ch in range(2)]
                for d in range(ND):
                    for ch in range(2):
                        mm(ps2[ch][:],
                           xt[:, d * S + j * P: d * S + (j + 1) * P],
                           wvt[:, d * D + ch * 512: d * D + (ch + 1) * 512],
                           start=(d == 0), stop=(d == ND - 1))
                for ch in range(2):
                    nc.scalar.copy(vres[:, j * D + ch * 512: j * D + (ch + 1) * 512], ps2[ch][:])

            # A4: wo -> woT (resident)
            for do in range(ND):
                wn = ldp2.tile([P, D], DT, tag="ld")
                nc.sync.dma_start(out=wn[:], in_=wo_in[do * P:(do + 1) * P, :])
                for vc in range(ND):
                    tp = tpps2.tile([P, P], DT, tag="tp")
                    nc.tensor.transpose(tp[:], wn[:, vc * P:(vc + 1) * P], ident[:])
                    nc.vector.tensor_copy(wot[:, vc * D + do * P: vc * D + (do + 1) * P], tp[:])

        # ---------------- Phase B ----------------
        with ExitStack() as pb:
            scps = pb.enter_context(tc.tile_pool(name="scps", bufs=2, space="PSUM"))
            ctxps = pb.enter_context(tc.tile_pool(name="ctxps", bufs=4, space="PSUM"))
            miscps = pb.enter_context(tc.tile_pool(name="miscps", bufs=1, space="PSUM"))
            outps = pb.enter_context(tc.tile_pool(name="outps", bufs=1, space="PSUM"))
            fbp = pb.enter_context(tc.tile_pool(name="fbp", bufs=9))
            expp = pb.enter_context(tc.tile_pool(name="expp", bufs=17))
            ctxsb = pb.enter_context(tc.tile_pool(name="ctxsb", bufs=8))
            outsb = pb.enter_context(tc.tile_pool(name="outsb", bufs=2))
            rsp = pb.enter_context(tc.tile_pool(name="rsp", bufs=2))

            for sb in range(NSB):
                fb = []
                for d2 in range(ND):
                    f = fbp.tile([P, SB], MM_DT, name=f"fb{d2}", tag="fb")
                    nc.sync.dma_start(out=f[:], in_=f_dram[d2 * P:(d2 + 1) * P, sb * SB:(sb + 1) * SB])
                    fb.append(f)

                # scoresT + exp per j-tile
                ets = []
                for j in range(NS):
                    sc = scps.tile([P, SB], F32, tag="sc")
                    for d2 in range(ND):
                        mm(sc[:],
                           xt[:, d2 * S + j * P: d2 * S + (j + 1) * P],
                           fb[d2][:],
                           start=(d2 == 0), stop=(d2 == ND - 1))
                    et = expp.tile([P, SB], MM_DT, name=f"et{j}", tag="et")
                    nc.scalar.activation(et[:], sc[:], EXP)
                    ets.append(et)

                # rowsums over j (partition dim) via ones-matmul
                rs = miscps.tile([1, SB], F32, tag="m")
                for j in range(NS):
                    mm(rs[:], ones[:, 0:1], ets[j][:], start=(j == 0), stop=(j == NS - 1))

                # ctxT in 2 passes of 4 vc-tiles (PSUM bank budget)
                ctx_tiles = [None] * ND
                for pss in range(2):
                    cps = [ctxps.tile([P, SB], F32, name=f"cps{vi}", tag="ctx") for vi in range(4)]
                    for j in range(NS):
                        for vi in range(4):
                            vc = pss * 4 + vi
                            mm(cps[vi][:],
                               vres[:, j * D + vc * P: j * D + (vc + 1) * P],
                               ets[j][:],
                               start=(j == 0), stop=(j == NS - 1))
                    for vi in range(4):
                        cs = ctxsb.tile([P, SB], MM_DT, name=f"cs{vi}", tag="cs")
                        nc.scalar.copy(cs[:], cps[vi][:])
                        ctx_tiles[pss * 4 + vi] = cs

                # reciprocal of rowsums -> per-partition [P,1] tiles
                rs_sb = rsp.tile([1, SB], DT, tag="rs")
                nc.vector.tensor_copy(rs_sb[:], rs[:])
                rc_sb = rsp.tile([1, SB], DT, tag="rc")
                nc.vector.reciprocal(rc_sb[:], rs_sb[:])
                recips = []
                for h in range(SB // P):
                    tp = miscps.tile([P, 1], F32, name=f"rtp{h}", tag="m")
                    nc.tensor.transpose(tp[:], rc_sb[:1, h * P:(h + 1) * P], ident[:1, :1])
                    rt = rsp.tile([P, 1], DT, name=f"rt{h}", tag="rt")
                    nc.vector.tensor_copy(rt[:], tp[:])
                    recips.append(rt)

                # final projection + normalization
                for it in range(SB // P):
                    for ch in range(2):
                        op = outps.tile([P, 512], F32, tag="op")
                        for vc in range(ND):
                            mm(op[:],
                               ctx_tiles[vc][:, it * P:(it + 1) * P],
                               wot[:, vc * D + ch * 512: vc * D + (ch + 1) * 512],
                               start=(vc == 0), stop=(vc == ND - 1))
                        ob = outsb.tile([P, 512], DT, tag="ob")
                        nc.scalar.activation(ob[:], op[:], COPY, scale=recips[it][:, 0:1])
                        nc.sync.dma_start(
                            out=out_d[(sb * (SB // P) + it) * P:(sb * (SB // P) + it + 1) * P,
                                      ch * 512:(ch + 1) * 512],
                            in_=ob[:])

    nc.compile()
    return nc


_NC_CACHE = None


def kernel(x, wq, wk, wv, wo):
    global _NC_CACHE
    if _NC_CACHE is None:
        _NC_CACHE = _build()
    nc = _NC_CACHE
    core_ids = list(range(N_CORES))
    in_maps = []
    for b in range(N_CORES):
        in_maps.append({
            "x": np.ascontiguousarray(x[b], dtype=np.float32),
            "wq": np.ascontiguousarray(wq, dtype=np.float32),
            "wk": np.ascontiguousarray(wk, dtype=np.float32),
            "wv": np.ascontiguousarray(wv, dtype=np.float32),
            "wo": np.ascontiguousarray(wo, dtype=np.float32),
        })
    res = run_bass_kernel_spmd(nc, in_maps, core_ids)
    return np.stack([res.results[b]["out"] for b in range(N_CORES)], axis=0)


# revision 9
# speedup vs baseline: 1.1541x; 1.1541x over previous
"""Single-head encoder attention block on 8 Trainium2 NeuronCores.

Math (per batch element b):
    q = x @ wq.T ; k = x @ wk.T ; v = x @ wv.T
    scores = (q @ k.T) / sqrt(1024) ; attn = softmax(scores, -1)
    out = (attn @ v) @ wo.T

Sharding: data-parallel over batch — batch 8 maps 1:1 onto the 8 cores;
weights replicated. No collectives.

Per-core algorithm (storage fp32 bits; matmul operands typed MM_DT):
  Two weight-product folds remove almost all operand transposes:
      scores = x (wq.T wk) x.T / 32          M  := wq.T @ wk
      attn @ v @ wo.T = attn @ x @ (wo wv).T U.T := wv.T-free form, built as
                                             UT[d,do] = sum_vc wv[vc,d] woT[vc,do]
  Phase A:
    xT  via identity-matmul transposes (resident)   [1024d, 2048s]
    M   = wq.T @ wk    (natural layouts)            [1024d1, 1024d2]
    F   = (M @ xT)/32  spilled to DRAM              [1024d2, 2048i]
    woT via identity-matmul transposes (half-passes)
    UT  = wv-nat x woT                              [1024d, 1024do]
    Z   = xT-stationary x UT (resident)             [2048j, 1024do]
  Phase B (per i-superblock of SB=512):
    scoresT[j,i] = sum_d2 xT[d2,j]*F[d2,i]; expT = exp(scoresT)
    rowsum over j via ones-matmul; reciprocal; transposed to per-partition
    out[i,do] = (sum_j expT[j,i-tile] * Z[j,do]) * recip[i]   (expT stationary)
"""

import os
import sys

for _p in ("/opt/trn_rl_repo", "/root/.axon_site/_ro/trn_rl_repo"):
    if os.path.isdir(_p) and _p not in sys.path:
        sys.path.insert(0, _p)

import numpy as np
from contextlib import ExitStack

import concourse.bacc as bacc
import concourse.tile as tile
from concourse import mybir, masks
from concourse.bass_utils import run_bass_kernel_spmd

P = 128
S = 2048          # sequence length (per core)
D = 1024          # model dim = dk = dv
NS = S // P       # 16 seq tiles
ND = D // P       # 8 dim tiles
SB = 512          # i-superblock width (query columns per block)
NSB = S // SB     # 4 superblocks
NIT = SB // P     # 4 i-tiles per superblock
SCALE = 1.0 / 32.0  # 1/sqrt(1024)
N_CORES = 8

DT = mybir.dt.float32
MM_DT = mybir.dt.float32r if os.environ.get("ENC_MM_DT", "f32r") == "f32r" else mybir.dt.float32
F32 = mybir.dt.float32
EXP = mybir.ActivationFunctionType.Exp
COPY = mybir.ActivationFunctionType.Copy


def _build():
    nc = bacc.Bacc("TRN2", target_bir_lowering=False, debug=False, num_devices=N_CORES)

    x_in = nc.dram_tensor("x", [S, D], DT, kind="ExternalInput").ap()
    wq_in = nc.dram_tensor("wq", [D, D], DT, kind="ExternalInput").ap()
    wk_in = nc.dram_tensor("wk", [D, D], DT, kind="ExternalInput").ap()
    wv_in = nc.dram_tensor("wv", [D, D], DT, kind="ExternalInput").ap()
    wo_in = nc.dram_tensor("wo", [D, D], DT, kind="ExternalInput").ap()
    out_d = nc.dram_tensor("out", [S, D], DT, kind="ExternalOutput").ap()
    f_dram = nc.dram_tensor("f_scratch", [D, S], MM_DT).ap()

    mm = nc.tensor.matmul

    with tile.TileContext(nc) as tc, ExitStack() as top:
        cst = top.enter_context(tc.tile_pool(name="cst", bufs=1))
        ident_f32 = cst.tile([P, P], DT)
        masks.make_identity(nc, ident_f32[:])
        ident = cst.tile([P, P], MM_DT)
        nc.vector.tensor_copy(ident[:], ident_f32[:])
        ones_f32 = cst.tile([P, 1], DT)
        nc.gpsimd.memset(ones_f32[:], 1.0)
        ones = cst.tile([P, 1], MM_DT)
        nc.vector.tensor_copy(ones[:], ones_f32[:])

        def tr(out_ap, in_ap):
            """out_ap[PSUM 128x128] = in_ap.T via normal matmul against identity."""
            mm(out_ap, in_ap, ident[:], start=True, stop=True)

        res1 = top.enter_context(tc.tile_pool(name="res1", bufs=1))
        xt = res1.tile([P, ND * S], MM_DT)    # xT: tile d -> [:, d*S:(d+1)*S] = [d-part, s]

        # ---------------- Phase A12: x transpose, M, F ----------------
        with ExitStack() as pa:
            tpps = pa.enter_context(tc.tile_pool(name="tpps", bufs=3, space="PSUM"))
            mmps = pa.enter_context(tc.tile_pool(name="mmps", bufs=5, space="PSUM"))
            ldp = pa.enter_context(tc.tile_pool(name="ldp", bufs=3))
            evp = pa.enter_context(tc.tile_pool(name="evp", bufs=4))
            wqk = pa.enter_context(tc.tile_pool(name="wqk", bufs=1))
            mwork = pa.enter_context(tc.tile_pool(name="mwork", bufs=1))

            # A0: load x row-tiles, transpose into xT
            for s in range(NS):
                xs = ldp.tile([P, D], MM_DT, tag="ld")
                nc.sync.dma_start(out=xs[:], in_=x_in[s * P:(s + 1) * P, :].bitcast(MM_DT))
                for d in range(ND):
                    tp = tpps.tile([P, P], F32, tag="tp")
                    tr(tp[:], xs[:, d * P:(d + 1) * P])
                    nc.vector.tensor_copy(xt[:, d * S + s * P: d * S + (s + 1) * P], tp[:])

            # A1: M = wq.T @ wk from natural layouts
            wqn = wqk.tile([P, ND * D], MM_DT)   # wq c-tile ct -> [:, ct*D:(ct+1)*D]
            wkn = wqk.tile([P, ND * D], MM_DT)
            mres = mwork.tile([P, ND * D], MM_DT)  # M d1-tile -> [:, d1*D:(d1+1)*D] = [d1-part, d2]
            for t in range(ND):
                nc.sync.dma_start(out=wqn[:, t * D:(t + 1) * D], in_=wq_in[t * P:(t + 1) * P, :].bitcast(MM_DT))
                nc.sync.dma_start(out=wkn[:, t * D:(t + 1) * D], in_=wk_in[t * P:(t + 1) * P, :].bitcast(MM_DT))
            for d1 in range(ND):
                for ch in range(2):
                    ps = mmps.tile([P, 512], F32, tag="mm")
                    for ct in range(ND):
                        mm(ps[:],
                           wqn[:, ct * D + d1 * P: ct * D + (d1 + 1) * P],
                           wkn[:, ct * D + ch * 512: ct * D + (ch + 1) * 512],
                           start=(ct == 0), stop=(ct == ND - 1))
                    nc.scalar.copy(mres[:, d1 * D + ch * 512: d1 * D + (ch + 1) * 512], ps[:])

            # A2: F[d2,i] = sum_d1 M[d1,d2] xT[d1,i], scaled by 1/32, spilled
            for d2 in range(ND):
                pss = [mmps.tile([P, 512], F32, name=f"fps{ic}", tag="mm") for ic in range(4)]
                for d1 in range(ND):
                    for ic in range(4):
                        mm(pss[ic][:],
                           mres[:, d1 * D + d2 * P: d1 * D + (d2 + 1) * P],
                           xt[:, d1 * S + ic * 512: d1 * S + (ic + 1) * 512],
                           start=(d1 == 0), stop=(d1 == ND - 1))
                for ic in range(4):
                    ev = evp.tile([P, 512], MM_DT, tag="ev")
                    nc.scalar.mul(ev[:], pss[ic][:], SCALE)
                    nc.sync.dma_start(out=f_dram[d2 * P:(d2 + 1) * P, ic * 512:(ic + 1) * 512], in_=ev[:])

        # Z resident for phase B (below the A3 transients on the pool stack)
        res2 = top.enter_context(tc.tile_pool(name="res2", bufs=1))
        zres = res2.tile([P, NS * D], MM_DT)  # Z: tile j -> [:, j*D:(j+1)*D] = [j-part, do]

        # ---------------- Phase A3: UT = wv.T-free fold, Z = x @ (wo wv).T ----
        # done in do-halves to bound SBUF: wot_h/ut_h are [*, 512] slices
        with ExitStack() as pw:
            tpps2 = pw.enter_context(tc.tile_pool(name="tpps2", bufs=3, space="PSUM"))
            mmps2 = pw.enter_context(tc.tile_pool(name="mmps2", bufs=5, space="PSUM"))
            ldp2 = pw.enter_context(tc.tile_pool(name="ldp2", bufs=2))
            wvp = pw.enter_context(tc.tile_pool(name="wvp", bufs=1))
            hwork = pw.enter_context(tc.tile_pool(name="hwork", bufs=1))

            wvn = wvp.tile([P, ND * D], MM_DT)   # wv natural: vc-tile t -> [:, t*D:(t+1)*D]
            for t in range(ND):
                nc.sync.dma_start(out=wvn[:, t * D:(t + 1) * D], in_=wv_in[t * P:(t + 1) * P, :].bitcast(MM_DT))

            for h in range(2):  # do-halves
                # woT half: [vc-part, 512 do-cols of half h]
                wot_h = hwork.tile([P, ND * 512], MM_DT, name=f"woth{h}", tag="wot")
                for dot in range(4):   # do-tiles within the half
                    wn = ldp2.tile([P, D], MM_DT, tag="ld")
                    do_row = h * 4 + dot
                    nc.sync.dma_start(out=wn[:], in_=wo_in[do_row * P:(do_row + 1) * P, :].bitcast(MM_DT))
                    for vc in range(ND):
                        tp = tpps2.tile([P, P], F32, tag="tp")
                        tr(tp[:], wn[:, vc * P:(vc + 1) * P])
                        nc.vector.tensor_copy(
                            wot_h[:, vc * 512 + dot * P: vc * 512 + (dot + 1) * P], tp[:])
                # UT half: [d-part, 512] per d-tile: sum_vc wv[vc,d] woT[vc,do_h]
                ut_h = hwork.tile([P, ND * 512], MM_DT, name=f"uth{h}", tag="ut")
                for d in range(ND):
                    ps = mmps2.tile([P, 512], F32, tag="mm")
                    for vc in range(ND):
                        mm(ps[:],
                           wvn[:, vc * D + d * P: vc * D + (d + 1) * P],
                           wot_h[:, vc * 512:(vc + 1) * 512],
                           start=(vc == 0), stop=(vc == ND - 1))
                    nc.scalar.copy(ut_h[:, d * 512:(d + 1) * 512], ps[:])
                # Z half: [j-part, do-half] = sum_d xT[d,j] UT[d,do_h]
                for j in range(NS):
                    ps = mmps2.tile([P, 512], F32, tag="mm")
                    for d in range(ND):
                        mm(ps[:],
                           xt[:, d * S + j * P: d * S + (j + 1) * P],
                           ut_h[:, d * 512:(d + 1) * 512],
                           start=(d == 0), stop=(d == ND - 1))
                    nc.scalar.copy(zres[:, j * D + h * 512: j * D + (h + 1) * 512], ps[:])

        # ---------------- Phase B ----------------
        with ExitStack() as pb:
            scps = pb.enter_context(tc.tile_pool(name="scps", bufs=3, space="PSUM"))
            outps = pb.enter_context(tc.tile_pool(name="outps", bufs=3, space="PSUM"))
            miscps = pb.enter_context(tc.tile_pool(name="miscps", bufs=2, space="PSUM"))
            fbp = pb.enter_context(tc.tile_pool(name="fbp", bufs=10))
            expp = pb.enter_context(tc.tile_pool(name="expp", bufs=16))
            outsb = pb.enter_context(tc.tile_pool(name="outsb", bufs=3))
            rsp = pb.enter_context(tc.tile_pool(name="rsp", bufs=2))
            rtp_pool = pb.enter_context(tc.tile_pool(name="rtp_pool", bufs=6))

            for sbi in range(NSB):
                fb = []
                for d2 in range(ND):
                    f = fbp.tile([P, SB], MM_DT, name=f"fb{d2}", tag="fb")
                    nc.sync.dma_start(out=f[:], in_=f_dram[d2 * P:(d2 + 1) * P, sbi * SB:(sbi + 1) * SB])
                    fb.append(f)

                # scoresT + exp per j-tile
                ets = []
                for j in range(NS):
                    sc = scps.tile([P, SB], F32, tag="sc")
                    for d2 in range(ND):
                        mm(sc[:],
                           xt[:, d2 * S + j * P: d2 * S + (j + 1) * P],
                           fb[d2][:],
                           start=(d2 == 0), stop=(d2 == ND - 1))
                    et = expp.tile([P, SB], MM_DT, name=f"et{j}", tag="et")
                    nc.scalar.activation(et[:], sc[:], EXP)
                    ets.append(et)

                # rowsums over j (partition dim) via ones-matmul
                rs = miscps.tile([1, SB], F32, tag="m")
                for j in range(NS):
                    mm(rs[:], ones[:, 0:1], ets[j][:], start=(j == 0), stop=(j == NS - 1))

                # reciprocal chain (DVE) — emitted early so it overlaps out-MMs
                rs_sb = rsp.tile([1, SB], DT, tag="rs")
                nc.vector.tensor_copy(rs_sb[:], rs[:])
                rc_sb = rsp.tile([1, SB], DT, tag="rc")
                nc.vector.reciprocal(rc_sb[:], rs_sb[:])

                # out[i,do] = sum_j expT[j, i-tile].T @ Z[j, do-chunk]; evict fused
                recips = [None] * NIT
                for gi in range(NIT * 2):
                    it, ch = gi // 2, gi % 2
                    op = outps.tile([P, 512], F32, name=f"op{ch}", tag="op")
                    for j in range(NS):
                        mm(op[:],
                           ets[j][:, it * P:(it + 1) * P],
                           zres[:, j * D + ch * 512: j * D + (ch + 1) * 512],
                           start=(j == 0), stop=(j == NS - 1))
                    if gi == 0:
                        # per-partition recip tiles via tiny PE transposes; PE
                        # reaches these after group 0 while DVE chain is done
                        for it2 in range(NIT):
                            tp = miscps.tile([P, 1], F32, name=f"rtp{it2}", tag="m")
                            nc.tensor.transpose(tp[:], rc_sb[:1, it2 * P:(it2 + 1) * P], ident_f32[:1, :1])
                            rt = rtp_pool.tile([P, 1], DT, name=f"rt{it2}", tag="rt")
                            nc.vector.tensor_copy(rt[:], tp[:])
                            recips[it2] = rt
                    ob = outsb.tile([P, 512], DT, tag="ob")
                    nc.scalar.activation(ob[:], op[:], COPY, scale=recips[it][:, 0:1])
                    nc.sync.dma_start(
                        out=out_d[(sbi * NIT + it) * P:(sbi * NIT + it + 1) * P,
                                  ch * 512:(ch + 1) * 512],
                        in_=ob[:])

    nc.compile()
    return nc


_NC_CACHE = None


def kernel(x, wq, wk, wv, wo):
    global _NC_CACHE
    if _NC_CACHE is None:
        _NC_CACHE = _build()
    nc = _NC_CACHE
    core_ids = list(range(N_CORES))
    in_maps = []
    for b in range(N_CORES):
        in_maps.append({
            "x": np.ascontiguousarray(x[b], dtype=np.float32),
            "wq": np.ascontiguousarray(wq, dtype=np.float32),
            "wk": np.ascontiguousarray(wk, dtype=np.float32),
            "wv": np.ascontiguousarray(wv, dtype=np.float32),
            "wo": np.ascontiguousarray(wo, dtype=np.float32),
        })
    res = run_bass_kernel_spmd(nc, in_maps, core_ids)
    return np.stack([res.results[b]["out"] for b in range(N_CORES)], axis=0)
